# revision 22
# baseline (speedup 1.0000x reference)
"""CtdetLoss (CenterNet detection loss) Bass kernel for 8 trn2 NeuronCores.

Strategy: pure data parallel over batch B=16 -> 2 batches per core.

Math restructuring (per o, b):
  The reference only ever consumes window (rectangle) sums of per-class maps:
    neg_sum[k] = A[k] - W1[k] + W2[k]
  with
    A[k]  = rectsum_k(S0),          S0 = sum_c neg0[c],  neg0 = ln(1-p)*p^2
    W1[k] = rectsum_k(neg0[c_k])
    W2[k] = rectsum_k(neg0[c_k] * (1-hm[c_k])^4)
    pos_sum[k] = sum over gt peaks in window of ln(p)*(1-p)^2
    num_pos[k] = # gt peaks in window  (pure host: hm is an input)
  wh/off losses only need out_wh/out_reg at the K object centers (pure host
  index gather, like the strip gathers).

  Device work per (o, bl):
   * Bulk A: stream out_hm (f16, host pre-transposed to contiguous
     [H, classes*W] chunks), ACT computes L = ln(1-p), one fused DVE
     scalar_tensor_tensor computes NG = (p pow 2) * L, TensorE accumulates
     8-class groups into psA[k, lane*W+x] with the 0/1 y-window mask wy as
     stationary weights; one fused DVE multiply+reduce against the (lane-
     replicated) x-window mask gives A[k].
   * W1/W2/pos: host gathers 20x20 window strips of out_hm around each
     object (pure indexing), pre-masked by the rect window (so no device
     mask needed); packed 2 partition rows per object -> [128, 200] tiles.
     Peaks for the pos term are host-gathered into a tiny [64, M] tile.
  Host only builds index gathers / 0-1 masks and does the final [O,B,K]
  combine + scalar reduction.
"""

import os
from contextlib import ExitStack

import numpy as np
import ml_dtypes

F16 = np.float16

O, B, C, H, W, K = 2, 16, 80, 128, 128, 64
HM_W, WH_W, OFF_W = 1.0, 0.1, 1.0
NCORES = 8
BL = B // NCORES  # batches per core
SH = 20  # strip height (max window height is exactly 20)
SW = 20  # strip width (max window width is exactly 20)
SR = SH // 2 * SW  # packed strip free size (200); 2 partition rows per object
M = 64  # max gt peaks per window (<= K objects of one class)
SSF = SR + M  # strip tile free size: [s_r | peaks-u]
CCH = 40  # out_hm classes per bulk chunk
NCH = C // CCH  # chunks per (o, bl)
CHF = CCH * W  # chunk free size (5120)
MMF = 4 * W  # classes per matmul slice * W = moving free size (512; 1 PSUM bank)
NMM = CHF // MMF  # matmul slices per chunk (5)
NSLOT = 16  # staging: 4 slots per (o, bl): A, W1, W2, P

_CACHE = {}


def _windows(wh, cxcy):
    """Window bounds per (b, k), mirroring the reference int arithmetic."""
    cx = cxcy[..., 0].astype(np.int64)
    cy = cxcy[..., 1].astype(np.int64)
    wpix = (wh[..., 0] * 0.5).astype(np.int32).astype(np.int64)
    hpix = (wh[..., 1] * 0.5).astype(np.int32).astype(np.int64)
    y0 = np.maximum(1, cy - hpix // 2 - 1)
    y1 = np.minimum(H - 1, cy + hpix // 2 + 1)
    x0 = np.maximum(1, cx - wpix // 2 - 1)
    x1 = np.minimum(W - 1, cx + wpix // 2 + 1)
    ys = np.minimum(y0, H - SH)  # strip start row (always fully in-bounds)
    xs = np.minimum(x0, W - SW)  # strip start col
    return y0, y1, x0, x1, ys, xs


def _pack_strip(a):
    """[.., K, SH, SW] -> packed [.., 128, SR]: obj k in rows k and k+64."""
    lead = a.shape[:-3]
    a = a.reshape(*lead, K, 2, SR)
    a = np.moveaxis(a, -2, -3)  # [.., 2, K, SR]
    return np.ascontiguousarray(a.reshape(*lead, 2 * K, SR))


def _build_core_inputs(out_hm, out_wh, out_reg, hm, wh, reg, cxcy, cls_idx):
    """Build per-core input dicts (host: pure indexing / 0-1 mask building)."""
    y0, y1, x0, x1, ys, xs = _windows(wh, cxcy)
    cls = cls_idx.astype(np.int64)
    bi = np.arange(B)[:, None]

    yy = np.arange(H)
    xx = np.arange(W)
    # [B, H, K] / [B, K, W] 0/1 window masks
    wy = ((yy[None, :, None] >= y0[:, None, :]) & (yy[None, :, None] < y1[:, None, :]))
    wxt = ((xx[None, None, :] >= x0[:, :, None]) & (xx[None, None, :] < x1[:, :, None]))
    wxt8 = np.tile(wxt.astype(F16), (1, 1, MMF // W))  # [B, K, 1024]

    # strip gathers: rows ys..ys+SH, cols xs..xs+SW of the object's class
    yg = ys[:, :, None] + np.arange(SH)[None, None, :]  # [B, K, SH]
    xg = xs[:, :, None] + np.arange(SW)[None, None, :]  # [B, K, SW]
    rect = (
        ((yg >= y0[:, :, None]) & (yg < y1[:, :, None]))[:, :, :, None]
        & ((xg >= x0[:, :, None]) & (xg < x1[:, :, None]))[:, :, None, :]
    )  # [B, K, SH, SW]

    def gather_strip(plane):  # plane [B, K, H, W] -> [B, K, SH, SW]
        g = np.take_along_axis(plane, yg[:, :, :, None], axis=2)
        return np.take_along_axis(g, xg[:, :, None, :], axis=3)

    shm_strip = gather_strip(hm[bi, cls])  # gt heatmap strip (f32)
    ispos = (shm_strip == 1.0) & rect
    numpos = ispos.sum((2, 3)).astype(np.float32)  # [B, K]
    u1 = 1.0 - shm_strip
    u2 = u1 * u1
    wr4 = np.where(rect, u2 * u2, 0.0).astype(np.float32)  # rect*(1-hm)^4
    wr4_p = _pack_strip(wr4.astype(F16))  # [B, 128, SR]

    # Guard: clamp to the largest f16 < 1 so ln(1-p) can never hit -inf
    # (reference clips p to 1-1e-4 anyway; inputs are < 0.999 so no-op).
    PMAX = np.float32(0.99902344)
    out_hm = np.minimum(out_hm, PMAX)

    # peak gather for the pos term: up to M peaks per window, pad u=0
    pk_b, pk_k, pk_r, pk_c = np.nonzero(ispos)
    cnt = np.zeros((B, K), np.int64)
    peak_u = np.zeros((O, B, K, M), np.float32)
    for b, k, r, c in zip(pk_b, pk_k, pk_r, pk_c):
        j = cnt[b, k]
        assert j < M
        cnt[b, k] = j + 1
        yy_, xx_ = ys[b, k] + r, xs[b, k] + c
        for o in range(O):
            peak_u[o, b, k, j] = 1.0 - out_hm[o, b, cls[b, k], yy_, xx_]

    # [O, B, 128, SSF] strip tiles: [rect-masked out_hm strip | peaks-u]
    ss = np.zeros((O, B, 2 * K, SSF), np.float32)
    for o in range(O):
        g = gather_strip(out_hm[o][bi, cls])  # [B, K, SH, SW]
        ss[o, :, :, :SR] = _pack_strip(np.where(rect, g, 0.0))
        ss[o, :, :K, SR:] = peak_u[o]

    # bulk layout: [O, B, NCH, H, CCH*W] contiguous
    ohm_t = np.ascontiguousarray(
        out_hm.reshape(O, B, NCH, CCH, H, W).transpose(0, 1, 2, 4, 3, 5)
    ).reshape(O, B, NCH, H, CHF)

    # host-side center gathers for wh/reg (pure indexing)
    cx = cxcy[..., 0].astype(np.int64)
    cy = cxcy[..., 1].astype(np.int64)
    pred_wh = np.empty((O, B, K, 2), np.float32)
    pred_rg = np.empty((O, B, K, 2), np.float32)
    for o in range(O):
        for ch in range(2):
            pred_wh[o, :, :, ch] = out_wh[o][bi, ch, cy, cx]
            pred_rg[o, :, :, ch] = out_reg[o][bi, ch, cy, cx]

    in_maps = []
    for core in range(NCORES):
        bs = slice(core * BL, (core + 1) * BL)
        in_maps.append(
            {
                "ohm": np.ascontiguousarray(ohm_t[:, bs]).astype(F16),
                "ss": np.ascontiguousarray(ss[:, bs]).astype(F16),
                "wr4": np.ascontiguousarray(wr4_p[bs]),
                "wy": np.ascontiguousarray(wy[bs]).astype(F16),
                "wxt": np.ascontiguousarray(wxt8[bs]),
            }
        )
    return in_maps, numpos, pred_wh, pred_rg


def build_bass():
    """Build the single SPMD Bass program (same for every core).

    DVE notes (HW-measured): full-tile offset-0 tensor_tensor ops average
    ~0.3 ns/elem (4x/2x mode mix); offset slices of wide tiles and all
    scalar_tensor_tensor ops run ~1 ns/elem. So the bulk uses two
    full-tile TTs (square in place, then multiply), and stt only for the
    small fused accumulate reduces.
    """
    import concourse.bass as bass  # noqa: F401
    import concourse.mybir as mybir
    import concourse.tile as tile
    from concourse import bacc

    f32 = mybir.dt.float32
    f16 = mybir.dt.float16
    AF = mybir.ActivationFunctionType
    OP = mybir.AluOpType

    nc = bacc.Bacc("TRN2", target_bir_lowering=False, debug=False,
                   num_devices=NCORES)

    ohm = nc.dram_tensor("ohm", [O, BL, NCH, H, CHF], f16, kind="ExternalInput")
    ssD = nc.dram_tensor("ss", [O, BL, 2 * K, SSF], f16, kind="ExternalInput")
    wr4D = nc.dram_tensor("wr4", [BL, 2 * K, SR], f16, kind="ExternalInput")
    wyD = nc.dram_tensor("wy", [BL, H, K], f16, kind="ExternalInput")
    wxtD = nc.dram_tensor("wxt", [BL, K, MMF], f16, kind="ExternalInput")
    res = nc.dram_tensor("res", [2 * K, NSLOT], f32, kind="ExternalOutput")

    def sq_mul(tt_eng, out_ap, p_ap, l_ap, accum=None):
        """out = p^2 * l (tt_eng: lp = l*p; DVE stt: out = lp*p [+accum]).

        Pool (gpsimd) supports tensor_tensor but NOT scalar_tensor_tensor
        on the TRN2 ISA, so the accumulating op always runs on DVE.
        """
        tt_eng.tensor_mul(out_ap, l_ap, p_ap)
        nc.vector.scalar_tensor_tensor(
            out=out_ap, in0=out_ap, scalar=1.0, in1=p_ap,
            op0=OP.mult, op1=OP.mult, accum_out=accum,
        )

    with tile.TileContext(nc) as tc, ExitStack() as ctx:
        const_pool = ctx.enter_context(tc.tile_pool(name="const", bufs=1))
        bulk_pool = ctx.enter_context(tc.tile_pool(name="bulk", bufs=2))
        strip_pool = ctx.enter_context(tc.tile_pool(name="strip", bufs=2))
        psum_pool = ctx.enter_context(tc.tile_pool(name="psum", bufs=2, space="PSUM"))

        staging = const_pool.tile([2 * K, NSLOT], f32, tag="staging")
        junkS = const_pool.tile([2 * K, SR], f16, tag="junkS")
        junkP = const_pool.tile([K, M], f16, tag="junkP")
        junkA = const_pool.tile([K, MMF], f16, tag="junkA")

        for bl in range(BL):
            # issue the first bulk chunk DMAs before anything else so the
            # ACT pipeline starts as early as possible
            pre = None
            if bl == 0:
                pre = []
                for ch in range(NCH):
                    p = bulk_pool.tile([H, CHF], f16, tag=f"pch{ch}",
                                       name=f"pch{ch}")
                    nc.sync.dma_start(p[:], ohm[0, 0, ch])
                    pre.append(p)
            wy_t = const_pool.tile([H, K], f16, tag=f"wy{bl}")
            nc.sync.dma_start(wy_t[:], wyD[bl])
            wxt_t = const_pool.tile([K, MMF], f16, tag=f"wxt{bl}")
            nc.sync.dma_start(wxt_t[:], wxtD[bl])
            wr4_t = const_pool.tile([2 * K, SR], f16, tag=f"wr4{bl}")
            nc.sync.dma_start(wr4_t[:], wr4D[bl])

            for o in range(O):
                col = (o * BL + bl) * 4

                # ---- bulk A: rectsum_k(S0) ----
                # Both chunks in flight; DVE ops interleaved so adjacent
                # instructions are independent (enables DVE co-issue).
                psA = psum_pool.tile([K, MMF], f32, tag="psA")
                pch, Lch, NG = [], [], []
                for ch in range(NCH):
                    if pre is not None and o == 0:
                        p = pre[ch]
                    else:
                        p = bulk_pool.tile([H, CHF], f16, tag=f"pch{ch}",
                                           name=f"pch{ch}")
                        nc.sync.dma_start(p[:], ohm[o, bl, ch])
                    pch.append(p)
                    Lch.append(bulk_pool.tile([H, CHF], f16, tag=f"Lch{ch}",
                                              name=f"Lch{ch}"))
                    NG.append(bulk_pool.tile([H, CHF], f16, tag=f"NG{ch}",
                                             name=f"NG{ch}"))
                # squares of chunk 0 on iterations 1,3 run on Pool to
                # balance DVE (Pool is otherwise idle)
                obl = bl * O + o
                for ch in range(NCH):
                    nc.scalar.activation(Lch[ch][:], pch[ch][:], AF.Ln,
                                         bias=1.0, scale=-1.0)
                    sq_eng = (nc.gpsimd if (ch == 0 and obl in (1, 3))
                              else nc.vector)
                    sq_eng.tensor_mul(NG[ch][:], pch[ch][:], pch[ch][:])
                for ch in range(NCH):
                    nc.vector.tensor_mul(NG[ch][:], NG[ch][:], Lch[ch][:])
                for ch in range(NCH):
                    for si in range(NMM):
                        nc.tensor.matmul(
                            psA[:],
                            wy_t[:],
                            NG[ch][:, si * MMF : (si + 1) * MMF],
                            start=(ch == 0 and si == 0),
                            stop=(ch == NCH - 1 and si == NMM - 1),
                        )
                # A[k] = sum(psA * wxt8) fused
                nc.vector.scalar_tensor_tensor(
                    out=junkA[:], in0=psA[:], scalar=1.0, in1=wxt_t[:],
                    op0=OP.mult, op1=OP.mult,
                    accum_out=staging[:K, col : col + 1],
                )

                # ---- strip terms: W1, W2, pos ----
                ss_t = strip_pool.tile([2 * K, SSF], f16, tag="ss")
                nc.sync.dma_start(ss_t[:], ssD[o, bl])
                LS = strip_pool.tile([2 * K, SSF], f16, tag="LS")
                nc.scalar.activation(LS[:], ss_t[:], AF.Ln, bias=1.0, scale=-1.0)
                ng0 = strip_pool.tile([2 * K, SR], f16, tag="ng0")
                # ng0 = s^2 * ln(1-s); W1 = sum(ng0) fused (all on Pool --
                # keeps DVE free for the bulk TT stream)
                sq_mul(nc.gpsimd, ng0[:], ss_t[:, :SR], LS[:, :SR],
                       accum=staging[:, col + 1 : col + 2])
                # W2 = sum(ng0 * wr4)
                nc.vector.scalar_tensor_tensor(
                    out=junkS[:], in0=ng0[:], scalar=1.0, in1=wr4_t[:],
                    op0=OP.mult, op1=OP.mult,
                    accum_out=staging[:, col + 2 : col + 3],
                )
                # pos = sum(u^2 * ln(1-u)) over host-gathered peaks (u=1-p)
                sq_mul(nc.gpsimd, junkP[:], ss_t[:K, SR:], LS[:K, SR:],
                       accum=staging[:K, col + 3 : col + 4])

        nc.sync.dma_start(res[:, :], staging[:])

    nc.compile()
    return nc


def _finalize(stats, numpos, pred_wh, pred_rg, wh, reg, reg_mask):
    """Combine per-core device stats into the 4 scalar losses (host)."""
    A = np.zeros((O, B, K), np.float32)
    W1 = np.zeros((O, B, K), np.float32)
    W2 = np.zeros((O, B, K), np.float32)
    possum = np.zeros((O, B, K), np.float32)
    for core in range(NCORES):
        r = np.asarray(stats[core], np.float32)  # [2K, NSLOT]
        lo, hi = r[:K], r[K:]
        for bl in range(BL):
            b = core * BL + bl
            for o in range(O):
                col = (o * BL + bl) * 4
                A[o, b] = lo[:, col]
                W1[o, b] = lo[:, col + 1] + hi[:, col + 1]
                W2[o, b] = lo[:, col + 2] + hi[:, col + 2]
                possum[o, b] = lo[:, col + 3]

    neg_sum = A - W1 + W2
    np_b = numpos[None]  # [1,B,K] broadcast over O
    hm_l = np.where(
        np_b > 0,
        -(possum + neg_sum) / np.maximum(np_b, 1.0),
        -neg_sum,
    ).astype(np.float32)
    wh_l = (np.abs(pred_wh - wh[None]).sum(-1) / np.float32(2.0 + 1e-4)).astype(
        np.float32
    )
    off_l = (np.abs(pred_rg - reg[None]).sum(-1) / np.float32(2.0 + 1e-4)).astype(
        np.float32
    )
    tot = (HM_W * hm_l + WH_W * wh_l + OFF_W * off_l).astype(np.float32)
    best = np.argmin(tot, axis=0)  # [B, K]

    def pick(a):
        return np.take_along_axis(a, best[None], axis=0)[0]

    m = reg_mask.astype(np.float32)
    loss = np.float32((pick(tot) * m).sum() / B)
    hm_loss = np.float32((pick(hm_l) * m).sum() / B)
    wh_loss = np.float32((pick(wh_l) * m).sum() / B)
    off_loss = np.float32((pick(off_l) * m).sum() / B)
    return (
        np.asarray(loss, np.float32),
        np.asarray(hm_loss, np.float32),
        np.asarray(wh_loss, np.float32),
        np.asarray(off_loss, np.float32),
    )


def _run_device(in_maps, trace=False):
    from concourse.bass_utils import run_bass_kernel_spmd

    if "nc" not in _CACHE:
        _CACHE["nc"] = build_bass()
    nc = _CACHE["nc"]
    kw = {}
    if trace:
        kw = dict(trace=True, trace_cores=list(range(NCORES)))
    r = run_bass_kernel_spmd(nc, in_maps, core_ids=list(range(NCORES)), **kw)
    return [out["res"] for out in r.results], r


def kernel(out_hm, out_wh, out_reg, hm, wh, reg, cxcy, cls_idx, ind, reg_mask):
    out_hm = np.asarray(out_hm, np.float32)
    out_wh = np.asarray(out_wh, np.float32)
    out_reg = np.asarray(out_reg, np.float32)
    hm = np.asarray(hm, np.float32)
    wh = np.asarray(wh, np.float32)
    reg = np.asarray(reg, np.float32)
    cxcy = np.asarray(cxcy)
    cls_idx = np.asarray(cls_idx)
    reg_mask = np.asarray(reg_mask)

    in_maps, numpos, pred_wh, pred_rg = _build_core_inputs(
        out_hm, out_wh, out_reg, hm, wh, reg, cxcy, cls_idx
    )
    trace = bool(int(os.environ.get("CTDET_TRACE", "0")))
    stats, _ = _run_device(in_maps, trace=trace)
    return _finalize(stats, numpos, pred_wh, pred_rg, wh, reg, reg_mask)


# revision 25
# speedup vs baseline: 1.1312x; 1.1312x over previous
"""CtdetLoss (CenterNet detection loss) Bass kernel for 8 trn2 NeuronCores.

Strategy: pure data parallel over batch B=16 -> 2 batches per core.

Math restructuring (per o, b):
  The reference only ever consumes window (rectangle) sums of per-class maps:
    neg_sum[k] = A[k] - W1[k] + W2[k]
  with
    A[k]  = rectsum_k(S0),          S0 = sum_c neg0[c],  neg0 = ln(1-p)*p^2
    W1[k] = rectsum_k(neg0[c_k])
    W2[k] = rectsum_k(neg0[c_k] * (1-hm[c_k])^4)
    pos_sum[k] = sum over gt peaks in window of ln(p)*(1-p)^2
    num_pos[k] = # gt peaks in window  (pure host: hm is an input)
  wh/off losses only need out_wh/out_reg at the K object centers (pure host
  index gather, like the strip gathers).

  Device work per (o, bl):
   * Bulk A: stream out_hm (f16, host pre-transposed to contiguous
     [H, classes*W] chunks), ACT computes L = ln(1-p), one fused DVE
     scalar_tensor_tensor computes NG = (p pow 2) * L, TensorE accumulates
     8-class groups into psA[k, lane*W+x] with the 0/1 y-window mask wy as
     stationary weights; one fused DVE multiply+reduce against the (lane-
     replicated) x-window mask gives A[k].
   * W1/W2/pos: host gathers 20x20 window strips of out_hm around each
     object (pure indexing), pre-masked by the rect window (so no device
     mask needed); packed 2 partition rows per object -> [128, 200] tiles.
     Peaks for the pos term are host-gathered into a tiny [64, M] tile.
  Host only builds index gathers / 0-1 masks and does the final [O,B,K]
  combine + scalar reduction.
"""

import os
from contextlib import ExitStack

import numpy as np
import ml_dtypes

F16 = np.float16

O, B, C, H, W, K = 2, 16, 80, 128, 128, 64
HM_W, WH_W, OFF_W = 1.0, 0.1, 1.0
NCORES = 8
BL = B // NCORES  # batches per core
SH = 20  # strip height (max window height is exactly 20)
SW = 20  # strip width (max window width is exactly 20)
SR = SH // 2 * SW  # packed strip free size (200); 2 partition rows per object
M = 64  # max gt peaks per window (<= K objects of one class)
SSF = SR + M  # strip tile free size: [s_r | peaks-u]
CCH = 40  # out_hm classes per bulk chunk
NCH = C // CCH  # chunks per (o, bl)
CHF = CCH * W  # chunk free size (5120)
MMF = 4 * W  # classes per matmul slice * W = moving free size (512; 1 PSUM bank)
NMM = CHF // MMF  # matmul slices per chunk (5)
NSLOT = 16  # staging: 4 slots per (o, bl): A, W1, W2, P

_CACHE = {}


def _windows(wh, cxcy):
    """Window bounds per (b, k), mirroring the reference int arithmetic."""
    cx = cxcy[..., 0].astype(np.int64)
    cy = cxcy[..., 1].astype(np.int64)
    wpix = (wh[..., 0] * 0.5).astype(np.int32).astype(np.int64)
    hpix = (wh[..., 1] * 0.5).astype(np.int32).astype(np.int64)
    y0 = np.maximum(1, cy - hpix // 2 - 1)
    y1 = np.minimum(H - 1, cy + hpix // 2 + 1)
    x0 = np.maximum(1, cx - wpix // 2 - 1)
    x1 = np.minimum(W - 1, cx + wpix // 2 + 1)
    ys = np.minimum(y0, H - SH)  # strip start row (always fully in-bounds)
    xs = np.minimum(x0, W - SW)  # strip start col
    return y0, y1, x0, x1, ys, xs


def _pack_strip(a):
    """[.., K, SH, SW] -> packed [.., 128, SR]: obj k in rows k and k+64."""
    lead = a.shape[:-3]
    a = a.reshape(*lead, K, 2, SR)
    a = np.moveaxis(a, -2, -3)  # [.., 2, K, SR]
    return np.ascontiguousarray(a.reshape(*lead, 2 * K, SR))


def _build_core_inputs(out_hm, out_wh, out_reg, hm, wh, reg, cxcy, cls_idx):
    """Build per-core input dicts (host: pure indexing / 0-1 mask building)."""
    y0, y1, x0, x1, ys, xs = _windows(wh, cxcy)
    cls = cls_idx.astype(np.int64)
    bi = np.arange(B)[:, None]

    yy = np.arange(H)
    xx = np.arange(W)
    # [B, H, K] / [B, K, W] 0/1 window masks
    wy = ((yy[None, :, None] >= y0[:, None, :]) & (yy[None, :, None] < y1[:, None, :]))
    wxt = ((xx[None, None, :] >= x0[:, :, None]) & (xx[None, None, :] < x1[:, :, None]))
    wxt8 = np.tile(wxt.astype(F16), (1, 1, MMF // W))  # [B, K, 1024]

    # strip gathers: rows ys..ys+SH, cols xs..xs+SW of the object's class
    yg = ys[:, :, None] + np.arange(SH)[None, None, :]  # [B, K, SH]
    xg = xs[:, :, None] + np.arange(SW)[None, None, :]  # [B, K, SW]
    rect = (
        ((yg >= y0[:, :, None]) & (yg < y1[:, :, None]))[:, :, :, None]
        & ((xg >= x0[:, :, None]) & (xg < x1[:, :, None]))[:, :, None, :]
    )  # [B, K, SH, SW]

    def gather_strip(plane):  # plane [B, K, H, W] -> [B, K, SH, SW]
        g = np.take_along_axis(plane, yg[:, :, :, None], axis=2)
        return np.take_along_axis(g, xg[:, :, None, :], axis=3)

    shm_strip = gather_strip(hm[bi, cls])  # gt heatmap strip (f32)
    ispos = (shm_strip == 1.0) & rect
    numpos = ispos.sum((2, 3)).astype(np.float32)  # [B, K]
    u1 = 1.0 - shm_strip
    u2 = u1 * u1
    wr4 = np.where(rect, u2 * u2, 0.0).astype(np.float32)  # rect*(1-hm)^4
    wr4_p = _pack_strip(wr4.astype(F16))  # [B, 128, SR]

    # Guard: clamp to the largest f16 < 1 so ln(1-p) can never hit -inf
    # (reference clips p to 1-1e-4 anyway; inputs are < 0.999 so no-op).
    PMAX = np.float32(0.99902344)
    out_hm = np.minimum(out_hm, PMAX)

    # peak gather for the pos term: up to M peaks per window, pad u=0
    pk_b, pk_k, pk_r, pk_c = np.nonzero(ispos)
    cnt = np.zeros((B, K), np.int64)
    peak_u = np.zeros((O, B, K, M), np.float32)
    for b, k, r, c in zip(pk_b, pk_k, pk_r, pk_c):
        j = cnt[b, k]
        assert j < M
        cnt[b, k] = j + 1
        yy_, xx_ = ys[b, k] + r, xs[b, k] + c
        for o in range(O):
            peak_u[o, b, k, j] = 1.0 - out_hm[o, b, cls[b, k], yy_, xx_]

    # [O, B, 128, SSF] strip tiles: [rect-masked out_hm strip | peaks-u]
    ss = np.zeros((O, B, 2 * K, SSF), np.float32)
    for o in range(O):
        g = gather_strip(out_hm[o][bi, cls])  # [B, K, SH, SW]
        ss[o, :, :, :SR] = _pack_strip(np.where(rect, g, 0.0))
        ss[o, :, :K, SR:] = peak_u[o]

    # bulk layout: [O, B, NCH, H, CCH*W] contiguous
    ohm_t = np.ascontiguousarray(
        out_hm.reshape(O, B, NCH, CCH, H, W).transpose(0, 1, 2, 4, 3, 5)
    ).reshape(O, B, NCH, H, CHF)

    # host-side center gathers for wh/reg (pure indexing)
    cx = cxcy[..., 0].astype(np.int64)
    cy = cxcy[..., 1].astype(np.int64)
    pred_wh = np.empty((O, B, K, 2), np.float32)
    pred_rg = np.empty((O, B, K, 2), np.float32)
    for o in range(O):
        for ch in range(2):
            pred_wh[o, :, :, ch] = out_wh[o][bi, ch, cy, cx]
            pred_rg[o, :, :, ch] = out_reg[o][bi, ch, cy, cx]

    in_maps = []
    for core in range(NCORES):
        bs = slice(core * BL, (core + 1) * BL)
        in_maps.append(
            {
                "ohm": np.ascontiguousarray(ohm_t[:, bs]).astype(F16),
                "ss": np.ascontiguousarray(ss[:, bs]).astype(F16),
                "wr4": np.ascontiguousarray(wr4_p[bs]),
                "wy": np.ascontiguousarray(wy[bs]).astype(F16),
                "wxt": np.ascontiguousarray(wxt8[bs]),
            }
        )
    return in_maps, numpos, pred_wh, pred_rg


def build_bass():
    """Build the single SPMD Bass program (same for every core).

    DVE notes (HW-measured): full-tile offset-0 tensor_tensor ops average
    ~0.3 ns/elem (4x/2x mode mix); offset slices of wide tiles and all
    scalar_tensor_tensor ops run ~1 ns/elem. So the bulk uses two
    full-tile TTs (square in place, then multiply), and stt only for the
    small fused accumulate reduces.
    """
    import concourse.bass as bass  # noqa: F401
    import concourse.mybir as mybir
    import concourse.tile as tile
    from concourse import bacc

    f32 = mybir.dt.float32
    f16 = mybir.dt.float16
    AF = mybir.ActivationFunctionType
    OP = mybir.AluOpType

    nc = bacc.Bacc("TRN2", target_bir_lowering=False, debug=False,
                   num_devices=NCORES)

    ohm = nc.dram_tensor("ohm", [O, BL, NCH, H, CHF], f16, kind="ExternalInput")
    ssD = nc.dram_tensor("ss", [O, BL, 2 * K, SSF], f16, kind="ExternalInput")
    wr4D = nc.dram_tensor("wr4", [BL, 2 * K, SR], f16, kind="ExternalInput")
    wyD = nc.dram_tensor("wy", [BL, H, K], f16, kind="ExternalInput")
    wxtD = nc.dram_tensor("wxt", [BL, K, MMF], f16, kind="ExternalInput")
    res = nc.dram_tensor("res", [2 * K, NSLOT], f32, kind="ExternalOutput")

    def sq_mul(tt_eng, out_ap, p_ap, l_ap, accum=None):
        """out = p^2 * l (tt_eng: lp = l*p; DVE stt: out = lp*p [+accum]).

        Pool (gpsimd) supports tensor_tensor but NOT scalar_tensor_tensor
        on the TRN2 ISA, so the accumulating op always runs on DVE.
        """
        tt_eng.tensor_mul(out_ap, l_ap, p_ap)
        nc.vector.scalar_tensor_tensor(
            out=out_ap, in0=out_ap, scalar=1.0, in1=p_ap,
            op0=OP.mult, op1=OP.mult, accum_out=accum,
        )

    with tile.TileContext(nc) as tc, ExitStack() as ctx:
        const_pool = ctx.enter_context(tc.tile_pool(name="const", bufs=1))
        bulk_pool = ctx.enter_context(tc.tile_pool(name="bulk", bufs=2))
        strip_pool = ctx.enter_context(tc.tile_pool(name="strip", bufs=2))
        psum_pool = ctx.enter_context(tc.tile_pool(name="psum", bufs=2, space="PSUM"))

        staging = const_pool.tile([2 * K, NSLOT], f32, tag="staging")
        junkS = const_pool.tile([2 * K, SR], f16, tag="junkS")
        junkP = const_pool.tile([K, M], f16, tag="junkP")
        junkA = const_pool.tile([K, MMF], f16, tag="junkA")

        for bl in range(BL):
            # issue the first bulk chunk DMAs before anything else so the
            # ACT pipeline starts as early as possible
            pre = None
            if bl == 0:
                pre = []
                for ch in range(NCH):
                    p = bulk_pool.tile([H, CHF], f16, tag=f"pch{ch}",
                                       name=f"pch{ch}")
                    nc.sync.dma_start(p[:], ohm[0, 0, ch])
                    pre.append(p)
            wy_t = const_pool.tile([H, K], f16, tag=f"wy{bl}")
            nc.sync.dma_start(wy_t[:], wyD[bl])
            wxt_t = const_pool.tile([K, MMF], f16, tag=f"wxt{bl}")
            nc.sync.dma_start(wxt_t[:], wxtD[bl])
            wr4_t = const_pool.tile([2 * K, SR], f16, tag=f"wr4{bl}")
            nc.sync.dma_start(wr4_t[:], wr4D[bl])

            for o in range(O):
                col = (o * BL + bl) * 4

                # ---- bulk A: rectsum_k(S0) ----
                # Both chunks in flight; DVE ops interleaved so adjacent
                # instructions are independent (enables DVE co-issue).
                psA = psum_pool.tile([K, MMF], f32, tag="psA")
                pch, Lch, NG = [], [], []
                for ch in range(NCH):
                    if pre is not None and o == 0:
                        p = pre[ch]
                    else:
                        p = bulk_pool.tile([H, CHF], f16, tag=f"pch{ch}",
                                           name=f"pch{ch}")
                        nc.sync.dma_start(p[:], ohm[o, bl, ch])
                    pch.append(p)
                    Lch.append(bulk_pool.tile([H, CHF], f16, tag=f"Lch{ch}",
                                              name=f"Lch{ch}"))
                    NG.append(bulk_pool.tile([H, CHF], f16, tag=f"NG{ch}",
                                             name=f"NG{ch}"))
                for ch in range(NCH):
                    nc.scalar.activation(Lch[ch][:], pch[ch][:], AF.Ln,
                                         bias=1.0, scale=-1.0)
                    nc.vector.tensor_mul(NG[ch][:], pch[ch][:], pch[ch][:])
                for ch in range(NCH):
                    nc.vector.tensor_mul(NG[ch][:], NG[ch][:], Lch[ch][:])
                for ch in range(NCH):
                    for si in range(NMM):
                        nc.tensor.matmul(
                            psA[:],
                            wy_t[:],
                            NG[ch][:, si * MMF : (si + 1) * MMF],
                            start=(ch == 0 and si == 0),
                            stop=(ch == NCH - 1 and si == NMM - 1),
                        )
                # A[k] = sum(psA * wxt8) fused
                nc.vector.scalar_tensor_tensor(
                    out=junkA[:], in0=psA[:], scalar=1.0, in1=wxt_t[:],
                    op0=OP.mult, op1=OP.mult,
                    accum_out=staging[:K, col : col + 1],
                )

                # ---- strip terms: W1, W2, pos ----
                ss_t = strip_pool.tile([2 * K, SSF], f16, tag="ss")
                nc.sync.dma_start(ss_t[:], ssD[o, bl])
                LS = strip_pool.tile([2 * K, SSF], f16, tag="LS")
                nc.scalar.activation(LS[:], ss_t[:], AF.Ln, bias=1.0, scale=-1.0)
                ng0 = strip_pool.tile([2 * K, SR], f16, tag="ng0")
                # ng0 = s^2 * ln(1-s); W1 = sum(ng0) fused (all on Pool --
                # keeps DVE free for the bulk TT stream)
                sq_mul(nc.vector, ng0[:], ss_t[:, :SR], LS[:, :SR],
                       accum=staging[:, col + 1 : col + 2])
                # W2 = sum(ng0 * wr4)
                nc.vector.scalar_tensor_tensor(
                    out=junkS[:], in0=ng0[:], scalar=1.0, in1=wr4_t[:],
                    op0=OP.mult, op1=OP.mult,
                    accum_out=staging[:, col + 2 : col + 3],
                )
                # pos = sum(u^2 * ln(1-u)) over host-gathered peaks (u=1-p)
                sq_mul(nc.vector, junkP[:], ss_t[:K, SR:], LS[:K, SR:],
                       accum=staging[:K, col + 3 : col + 4])

        nc.sync.dma_start(res[:, :], staging[:])

    nc.compile()
    return nc


def _finalize(stats, numpos, pred_wh, pred_rg, wh, reg, reg_mask):
    """Combine per-core device stats into the 4 scalar losses (host)."""
    A = np.zeros((O, B, K), np.float32)
    W1 = np.zeros((O, B, K), np.float32)
    W2 = np.zeros((O, B, K), np.float32)
    possum = np.zeros((O, B, K), np.float32)
    for core in range(NCORES):
        r = np.asarray(stats[core], np.float32)  # [2K, NSLOT]
        lo, hi = r[:K], r[K:]
        for bl in range(BL):
            b = core * BL + bl
            for o in range(O):
                col = (o * BL + bl) * 4
                A[o, b] = lo[:, col]
                W1[o, b] = lo[:, col + 1] + hi[:, col + 1]
                W2[o, b] = lo[:, col + 2] + hi[:, col + 2]
                possum[o, b] = lo[:, col + 3]

    neg_sum = A - W1 + W2
    np_b = numpos[None]  # [1,B,K] broadcast over O
    hm_l = np.where(
        np_b > 0,
        -(possum + neg_sum) / np.maximum(np_b, 1.0),
        -neg_sum,
    ).astype(np.float32)
    wh_l = (np.abs(pred_wh - wh[None]).sum(-1) / np.float32(2.0 + 1e-4)).astype(
        np.float32
    )
    off_l = (np.abs(pred_rg - reg[None]).sum(-1) / np.float32(2.0 + 1e-4)).astype(
        np.float32
    )
    tot = (HM_W * hm_l + WH_W * wh_l + OFF_W * off_l).astype(np.float32)
    best = np.argmin(tot, axis=0)  # [B, K]

    def pick(a):
        return np.take_along_axis(a, best[None], axis=0)[0]

    m = reg_mask.astype(np.float32)
    loss = np.float32((pick(tot) * m).sum() / B)
    hm_loss = np.float32((pick(hm_l) * m).sum() / B)
    wh_loss = np.float32((pick(wh_l) * m).sum() / B)
    off_loss = np.float32((pick(off_l) * m).sum() / B)
    return (
        np.asarray(loss, np.float32),
        np.asarray(hm_loss, np.float32),
        np.asarray(wh_loss, np.float32),
        np.asarray(off_loss, np.float32),
    )


def _run_device(in_maps, trace=False):
    from concourse.bass_utils import run_bass_kernel_spmd

    if "nc" not in _CACHE:
        _CACHE["nc"] = build_bass()
    nc = _CACHE["nc"]
    kw = {}
    if trace:
        kw = dict(trace=True, trace_cores=list(range(NCORES)))
    r = run_bass_kernel_spmd(nc, in_maps, core_ids=list(range(NCORES)), **kw)
    return [out["res"] for out in r.results], r


def kernel(out_hm, out_wh, out_reg, hm, wh, reg, cxcy, cls_idx, ind, reg_mask):
    out_hm = np.asarray(out_hm, np.float32)
    out_wh = np.asarray(out_wh, np.float32)
    out_reg = np.asarray(out_reg, np.float32)
    hm = np.asarray(hm, np.float32)
    wh = np.asarray(wh, np.float32)
    reg = np.asarray(reg, np.float32)
    cxcy = np.asarray(cxcy)
    cls_idx = np.asarray(cls_idx)
    reg_mask = np.asarray(reg_mask)

    in_maps, numpos, pred_wh, pred_rg = _build_core_inputs(
        out_hm, out_wh, out_reg, hm, wh, reg, cxcy, cls_idx
    )
    trace = bool(int(os.environ.get("CTDET_TRACE", "0")))
    stats, _ = _run_device(in_maps, trace=trace)
    return _finalize(stats, numpos, pred_wh, pred_rg, wh, reg, reg_mask)
